# revision 36
# baseline (speedup 1.0000x reference)
"""Distributed GQA attention kernel for 8 TRN2 NeuronCores.

Strategy (tensor-parallel over heads, A2A re-shard before o_proj):
  - Core i owns q heads 4i..4i+3 and kv head i (GQA group) -> attention is
    fully local per core.
  - QKV projection computed TRANSPOSED (features on partitions):
      qkvT[f, s] = w_qkv_shard @ hidden.T
    Input loads are split across the sync/gpsimd/scalar DMA queues;
    PSUM->SBUF staging runs on the (otherwise idle) ACT engine.
  - RoPE (neox) applied in [d, s] layout via elementwise DVE ops.
  - Attention in "scoresT" layout: scoresT[k, q] = kT.T @ qT. Softmax without
    max-subtraction; exp on ACT; causal masking via 0/1 mask multiply after
    exp on diagonal blocks. Score PSUM cycles through THREE round-robin
    2-bank buffers so the exp(g) -> scores(g+3) WAR never serializes the PE
    behind the ACT engine (the attention phase is exp-rate-bound). PV matmul
    with ones-augmented v gives ctxT and softmax denominators in one
    accumulation.
  - Per (jc, head): one fast PSUM->SBUF copy frees the ctx banks; the
    normalize chain (reciprocal, gpsimd partition-broadcast, multiply,
    scatter) is DEFERRED one jc so it executes in vector-queue slack and its
    cross-engine latency never head-of-line blocks the next jc's mask
    multiplies (which gate the PE's PV matmuls).
  - Two AllToAlls (one per head pair) re-shard ctxT from head-sharded to
    sequence-sharded; A2A-0 overlaps pair-1 attention. An input-free warm-up
    collective (zeros baked into the NEFF) triggers at kernel start and
    absorbs the ncfw cold-start delay off the critical path.
  - o_proj locally on own 256 seq rows with full w_o.T, split into an even-kt
    pass (only needs A2A-0) that hides under the second AllToAll, then an
    odd-kt pass. w_o streams through a half-size SBUF buffer allocated in the
    space freed by the h tiles (even-kt cols first; odd-kt cols overwrite
    after the even-pass reads retire). Output staged bf16 on the ACT engine,
    row-sharded; host concatenates and converts to f32.
"""

import os
import numpy as np
import ml_dtypes

import concourse.bass as bass
import concourse.mybir as mybir
from concourse import bacc, tile

F32 = mybir.dt.float32
BF16 = mybir.dt.bfloat16
BF16_NP = ml_dtypes.bfloat16

# Problem constants (hardcoded per harness contract)
H = 2048
S = 2048
NH = 32
NKV = 8
HD = 64
Q_SIZE = NH * HD      # 2048
KV_SIZE = NKV * HD    # 512
NCORES = 8
QH = NH // NCORES     # 4 q heads per core
ROPE_THETA = 10000.0

P = 128
KT = H // P           # 16 contraction tiles over H
NQ = 512              # query chunk (matmul moving free dim)
NJC = S // NQ         # 4 query chunks
NKB = S // P          # 16 key tiles
SROWS = S // NCORES   # 256 seq rows per core after A2A

_NC_CACHE = None
LAST_RESULTS = None


def _build_nc():
    nc = bacc.Bacc(
        "TRN2",
        target_bir_lowering=False,
        debug=False,
        num_devices=NCORES,
    )

    # ---- I/O ----
    hT_d = nc.dram_tensor("hT", [P, KT * S], BF16, kind="ExternalInput")
    wq_d = nc.dram_tensor("wqkvT", [P, KT * 384], BF16, kind="ExternalInput")
    wo_d = nc.dram_tensor("woT", [P, KT * H], BF16, kind="ExternalInput")
    ropec_d = nc.dram_tensor("ropec", [P, S], BF16, kind="ExternalInput")
    ropes_d = nc.dram_tensor("ropes", [P, S], BF16, kind="ExternalInput")
    masks_d = nc.dram_tensor("masks", [P, P], BF16, kind="ExternalInput")
    ident_d = nc.dram_tensor("ident", [P, P], F32, kind="ExternalInput")
    out_d = nc.dram_tensor("out", [SROWS, H], BF16, kind="ExternalOutput")

    rg = [list(range(NCORES))]

    with tile.TileContext(nc) as tc:
        with (
            tc.tile_pool(name="dram", bufs=1, space="DRAM") as dram,
            tc.tile_pool(name="const", bufs=1) as const,
            tc.tile_pool(name="qk", bufs=1) as qkpool,
            tc.tile_pool(name="esb", bufs=4) as esb,
            tc.tile_pool(name="small", bufs=2) as small,
            tc.tile_pool(name="cstp", bufs=4) as cstp,
            tc.tile_pool(name="outp", bufs=2) as outp,
        ):
            # A2A buffers, one per head pair (shard j rows = my pair ctxT for
            # seq cols of core j)
            cc_in = [
                dram.tile([NCORES * P, SROWS], BF16, tag=f"cc_in{p}", name=f"cc_in{p}")
                for p in range(2)
            ]
            cc_out = [
                dram.tile([NCORES * P, SROWS], BF16, tag=f"cc_out{p}", name=f"cc_out{p}")
                for p in range(2)
            ]

            # input-free warm-up collective: zeros baked into the NEFF, so no
            # SBUF staging or DMA dep -- it triggers immediately at kernel
            # start and absorbs the ncfw cold-start delay off the critical path
            ccw_in = nc.inline_tensor(
                np.zeros((NCORES, 16), dtype=BF16_NP), name="ccw_in"
            )
            ccw_out = dram.tile([NCORES, 16], BF16, tag="ccw_out")
            nc.gpsimd.collective_compute(
                "AllToAll",
                mybir.AluOpType.bypass,
                replica_groups=rg,
                ins=[ccw_in[:, :]],
                outs=[ccw_out.opt()],
            )

            qpair = [
                qkpool.tile([P, S], BF16, tag=f"qpair{m}", name=f"qpair{m}")
                for m in range(2)
            ]
            kdup = qkpool.tile([P, S], BF16, tag="kdup")
            # v_aug per key tile kb: cols [kb*128, kb*128+64) = ones (the PV
            # matmul then lands the softmax denominators REPLICATED on out
            # partitions 0:64 -- no gpsimd partition_broadcast needed in the
            # normalize chain), cols [kb*128+64, kb*128+128) = vT.
            v_aug = qkpool.tile([P, NKB * P], BF16, tag="v_aug")
            cc_sb = qkpool.tile([P, KT * SROWS], BF16, tag="cc_sb")

            # ========== Phase 1: QKV projection (transposed) + RoPE ==========
            # hT is staged n-major on the host ([P, n * kt * 512]) so the
            # query-chunk-n columns of ALL kt tiles are one contiguous DRAM
            # block: few big DMAs, issued in dependency-priority order across
            # the three DMA-capable queues (each dma_start costs ~0.7us of
            # descriptor-gen on its queue sequencer, so count matters).  The
            # kt0-1 pieces of wq/h are split off tiny so the first matmuls
            # aren't stuck behind megabyte transfers sharing the DMA engines.
            # Phase-1 tiles live in their own pool, released after phase 1 so
            # the full-size w_o buffer reuses their SBUF.
            ph1_ctx = tc.tile_pool(name="ph1", bufs=1)
            ph1 = ph1_ctx.__enter__()
            wq_sb = ph1.tile([P, KT * 384], BF16, tag="wq_sb")
            h_n = []
            for n in range(NJC):
                hn = ph1.tile([P, KT * NQ], BF16, tag=f"h{n}", name=f"h{n}")
                h_n.append(hn)
            ident = const.tile([P, P], F32, tag="ident")
            masks = const.tile([P, P], BF16, tag="masks")
            ropec = const.tile([P, S], BF16, tag="ropec")
            ropes = const.tile([P, S], BF16, tag="ropes")

            HB = KT * NQ  # 8192 cols per n-block of hT_d

            def _h_dma(q, n, lo, hi):
                q.dma_start(
                    h_n[n][:, lo * NQ : hi * NQ],
                    hT_d[:, n * HB + lo * NQ : n * HB + hi * NQ],
                )

            # wave 1 (first 4 proj matmuls' inputs, tiny, at the head of ALL
            # three queues so no big transfer's descriptors get ahead)
            nc.sync.dma_start(wq_sb[:, 0:768], wq_d[:, 0:768])
            _h_dma(nc.scalar, 0, 0, 2)
            _h_dma(nc.gpsimd, 0, 2, 4)
            # wave 2 (rest of group(2,0) inputs)
            nc.sync.dma_start(wq_sb[:, 768:3072], wq_d[:, 768:3072])
            _h_dma(nc.scalar, 0, 4, 10)
            _h_dma(nc.gpsimd, 0, 10, 16)
            # wave 3
            nc.sync.dma_start(wq_sb[:, 3072:6144], wq_d[:, 3072:6144])
            nc.scalar.dma_start(ropec[:, 0:NQ], ropec_d[:, 0:NQ])
            nc.gpsimd.dma_start(ropes[:, 0:NQ], ropes_d[:, 0:NQ])
            # wave 4+: later n blocks and tables in need order
            _h_dma(nc.sync, 1, 0, 8)
            _h_dma(nc.scalar, 1, 8, 16)
            nc.gpsimd.dma_start(ident[:], ident_d[:])
            nc.gpsimd.dma_start(ropec[:, NQ:S], ropec_d[:, NQ:S])
            nc.gpsimd.dma_start(ropes[:, NQ:S], ropes_d[:, NQ:S])
            nc.gpsimd.dma_start(masks[:], masks_d[:])
            # h2/h3 throttled behind a pacer DMA that depends on the n=0
            # proj staging: their ~4MB would otherwise contend with h0/h1 in
            # the (device-HBM-saturated) early window and stall the PE
            pacer_d = dram.tile([1, 16], F32, tag="pacer_d")

            # proj staging, one per group: all three groups accumulate
            # CONCURRENTLY per kt so each h chunk feeds ~0.8us of PE work per
            # 128KB -- the PE tracks the (device-HBM-saturated) DMA instead
            # of burning through one group and stalling
            kv_sb = ph1.tile([P, S], F32, tag="kv_sb")
            q0_sb = ph1.tile([P, S], F32, tag="q0_sb")
            q1_sb = ph1.tile([P, S], F32, tag="q1_sb")

            with (
                tc.tile_pool(name="ropetmp", bufs=1) as ropetmp,
                tc.tile_pool(name="ps_mm", bufs=4, space="PSUM") as ps_mm,
                tc.tile_pool(name="ps_vt", bufs=2, space="PSUM") as ps_vt,
            ):
                def emit_rope(src_sb, dst, n, nrows):
                    # rope one NQ chunk of src_sb [d, s] layout into dst (bf16)
                    c0, c1 = NQ * n, NQ * (n + 1)
                    swp = ropetmp.tile([P, NQ], F32, tag="swp", name="swp")
                    nb = nrows // 64
                    for b in range(nb):
                        o = 64 * b
                        nc.vector.tensor_copy(
                            swp[o : o + 32, :], src_sb[o + 32 : o + 64, c0:c1]
                        )
                        nc.vector.tensor_copy(
                            swp[o + 32 : o + 64, :], src_sb[o : o + 32, c0:c1]
                        )
                    prod = ropetmp.tile([P, NQ], F32, tag="prod", name="prod")
                    nc.vector.tensor_mul(
                        prod[0:nrows, :], src_sb[0:nrows, c0:c1], ropec[0:nrows, c0:c1]
                    )
                    prod2 = ropetmp.tile([P, NQ], F32, tag="prod2", name="prod2")
                    nc.vector.tensor_mul(
                        prod2[0:nrows, :], swp[0:nrows, :], ropes[0:nrows, c0:c1]
                    )
                    if nrows == 64:
                        # k: write roped rows duplicated into both kdup halves
                        nc.vector.tensor_add(
                            dst[0:64, c0:c1], prod[0:64, :], prod2[0:64, :]
                        )
                        nc.vector.tensor_add(
                            dst[64:128, c0:c1], prod[0:64, :], prod2[0:64, :]
                        )
                    else:
                        nc.vector.tensor_add(dst[:, c0:c1], prod[:], prod2[:])

                # ones blocks of v_aug (single strided memset)
                nc.vector.memset(
                    v_aug[:].rearrange("p (kb c) -> p kb c", kb=NKB)[:, :, 0:64],
                    1.0,
                )
                # per-n, all 3 groups accumulate concurrently kt-by-kt (3
                # PSUM chains, 2-buf rings per group tag = 6 banks + vt 2)
                dst_sb = {2: kv_sb, 0: q0_sb, 1: q1_sb}
                for n in range(NJC):
                    pss = {
                        m: ps_mm.tile(
                            [P, NQ], F32, tag=f"mm{m}", name=f"mm{m}", bufs=2
                        )
                        for m in (2, 0, 1)
                    }
                    for kt in range(KT):
                        for m in (2, 0, 1):
                            nc.tensor.matmul(
                                pss[m][:],
                                wq_sb[
                                    :, kt * 384 + 128 * m : kt * 384 + 128 * (m + 1)
                                ],
                                h_n[n][:, NQ * kt : NQ * (kt + 1)],
                                start=(kt == 0),
                                stop=(kt == KT - 1),
                            )
                    # PSUM -> SBUF staging on the (idle in phase 1) ACT engine
                    for m in (2, 0, 1):
                        nc.scalar.copy(
                            dst_sb[m][:, NQ * n : NQ * (n + 1)], pss[m][:]
                        )
                    if n == 0:
                        # pacers: h2/h3 wait for the n=0 staging copy
                        nc.sync.dma_start(pacer_d[:, 0:8], kv_sb[0:1, 0:8])
                        nc.scalar.dma_start(pacer_d[:, 8:16], kv_sb[0:1, 8:16])
                        _h_dma(nc.sync, 2, 0, 8)
                        _h_dma(nc.scalar, 2, 8, 16)
                        _h_dma(nc.sync, 3, 0, 8)
                        _h_dma(nc.scalar, 3, 8, 16)
                    emit_rope(kv_sb, kdup, n, 64)
                    # transpose v chunk -> v_aug [keys, ones(64) | vT(64)]
                    for kb in range(4 * n, 4 * n + 4):
                        vps = ps_vt.tile([P, 64], F32, tag="vt")
                        nc.tensor.transpose(
                            vps[:, 0:64],
                            kv_sb[64:128, P * kb : P * (kb + 1)],
                            ident[64:128, 64:128],
                        )
                        nc.vector.tensor_copy(
                            v_aug[:, kb * P + 64 : kb * P + 128], vps[:, 0:64]
                        )
                    emit_rope(q0_sb, qpair[0], n, 128)
                    emit_rope(q1_sb, qpair[1], n, 128)

            # phase-1 SBUF released; w_o gets FULL-size tiles in that space
            # (loads overlap attention on the gpsimd queue; no odd-kt refill
            # on the post-A2A critical tail)
            ph1_ctx.__exit__(None, None, None)
            wo_ctx = tc.tile_pool(name="wo_full", bufs=1)
            wo_pool = wo_ctx.__enter__()
            wo_tiles = []
            for n in range(NJC):
                wos = wo_pool.tile([P, KT * NQ], BF16, tag=f"wos{n}", name=f"wos{n}")
                wo_tiles.append(wos)
            for n in range(NJC):
                nc.gpsimd.dma_start(
                    wo_tiles[n][:].rearrange("p (kt c) -> p kt c", kt=KT),
                    wo_d[:].rearrange("p (kt c) -> p kt c", kt=KT)[
                        :, :, NQ * n : NQ * (n + 1)
                    ],
                )

            # ========== Phase 2: attention ==========
            cc_insts = []
            # two round-robin 2-bank score-PSUM streams (exp(g) ->
            # scores(g+2) WAR, ~2 pipeline periods of slack) + DOUBLE-buffered
            # ctx banks: each jc's first PV writes fresh banks instead of
            # waiting on the previous jc's PSUM->SBUF copies (that WAR cost
            # ~2us of PE idle at every jc boundary).
            ps_s_ctx = tc.tile_pool(name="ps_s", bufs=2, space="PSUM")
            ps_s = ps_s_ctx.__enter__()
            ps_ctx_ctx = tc.tile_pool(name="ps_ctx", bufs=2, space="PSUM")
            ps_ctx = ps_ctx_ctx.__enter__()

            def emit_pair(p):
                pending = []
                # jc=1 first: its 4 off-diagonal chunks need no mask multiply,
                # giving the PE a mask-free runway while the vector queue
                # drains the tail of the previous phase's work
                for jc in (1, 0, 2, 3):
                    nkb = 4 * (jc + 1)
                    ctxs = [
                        ps_ctx.tile([P, NQ], F32, tag=f"ctx{hh}", name=f"ctx{hh}")
                        for hh in range(2)
                    ]
                    for kb in range(nkb):
                        d = kb - 4 * jc
                        # diagonal chunks: q cols < 128*d are fully masked --
                        # trim them from the score matmul, exp, and PV
                        q0 = P * d if d > 0 else 0
                        sp = ps_s.tile([P, 2 * NQ], F32, tag="sp", name="sp")
                        for hh in range(2):
                            base = 64 * hh
                            nc.tensor.matmul(
                                sp[:, NQ * hh + q0 : NQ * (hh + 1)],
                                kdup[base : base + 64, P * kb : P * (kb + 1)],
                                qpair[p][
                                    base : base + 64,
                                    NQ * jc + q0 : NQ * (jc + 1),
                                ],
                                start=True,
                                stop=True,
                            )
                        e = esb.tile([P, 2 * NQ], BF16, tag="e", name="e")
                        if q0:
                            sp3 = sp[:].rearrange("p (h q) -> p h q", h=2)[
                                :, :, q0:NQ
                            ]
                            e3 = e[:].rearrange("p (h q) -> p h q", h=2)[
                                :, :, q0:NQ
                            ]
                            nc.scalar.activation(
                                e3, sp3, mybir.ActivationFunctionType.Exp,
                                scale=0.125,
                            )
                        else:
                            nc.scalar.activation(
                                e[:], sp[:], mybir.ActivationFunctionType.Exp,
                                scale=0.125,
                            )
                        if d >= 0:
                            # only the leading 128-col block of the trimmed
                            # range is partial; mask is the same [128, 128]
                            # triangle for every (jc, d)
                            for hh in range(2):
                                nc.vector.tensor_mul(
                                    e[:, NQ * hh + q0 : NQ * hh + q0 + P],
                                    e[:, NQ * hh + q0 : NQ * hh + q0 + P],
                                    masks[:],
                                )
                        for hh in range(2):
                            nc.tensor.matmul(
                                ctxs[hh][:, q0:NQ],
                                v_aug[:, kb * P : (kb + 1) * P],
                                e[:, NQ * hh + q0 : NQ * (hh + 1)],
                                start=(kb == 0),
                                stop=(kb == nkb - 1),
                            )
                    # fast PSUM->SBUF copies free the ctx banks for the next jc:
                    # sums (replicated on partitions 0:64 by the ones-first
                    # v_aug) and ctx values (64:128) each staged to base-0
                    # tiles -- the HW verifier requires tensor-tensor INPUTS
                    # on the same partitions (outputs may shift).  The
                    # normalize+scatter chain is deferred one jc so it
                    # executes in vector-queue slack and never blocks the
                    # next jc's masks/PVs.
                    csts = []
                    for hh in range(2):
                        cst_s = cstp.tile([64, NQ], F32, tag="cst_s", name="cst_s")
                        nc.vector.tensor_copy(cst_s[:], ctxs[hh][0:64, :])
                        cst_c = cstp.tile([64, NQ], F32, tag="cst_c", name="cst_c")
                        nc.vector.tensor_copy(cst_c[:], ctxs[hh][64:128, :])
                        csts.append((cst_s, cst_c))

                    def chain(jc=jc, csts=csts):
                        # both heads land in one [128, NQ] tile so each dest
                        # shard is a single scatter DMA (each dma_start costs
                        # ~0.7us of descriptor-gen on the queue sequencer)
                        ctxn = small.tile([P, NQ], BF16, tag="ctxn")
                        for hh in range(2):
                            cst_s, cst_c = csts[hh]
                            rec = small.tile([64, NQ], F32, tag="rec")
                            nc.vector.reciprocal_approx_fast(rec[:], cst_s[:])
                            nc.vector.tensor_mul(
                                ctxn[64 * hh : 64 * (hh + 1), :],
                                cst_c[:],
                                rec[:],
                            )
                        # scatter: shard j (rows 128j..) holds my pair-p ctxT
                        # for core j's seq cols
                        for half in range(2):
                            j = 2 * jc + half
                            nc.sync.dma_start(
                                cc_in[p][P * j : P * (j + 1), :],
                                ctxn[:, SROWS * half : SROWS * (half + 1)],
                            )

                    if pending:
                        pending.pop(0)()
                    pending.append(chain)
                while pending:
                    pending.pop(0)()
                # A2A for this pair; pair 0's collective overlaps pair 1's
                # attention.
                cc_insts.append(
                    nc.gpsimd.collective_compute(
                        "AllToAll",
                        mybir.AluOpType.bypass,
                        replica_groups=rg,
                        ins=[cc_in[p].opt()],
                        outs=[cc_out[p].opt()],
                    )
                )

            emit_pair(0)
            emit_pair(1)

            ps_ctx_ctx.__exit__(None, None, None)
            ps_s_ctx.__exit__(None, None, None)

            # ========== Phase 3: o_proj on own seq rows ==========
            # qd chunk (2j + p) <- cc_out[p] rows [128j .. 128j+128); all 8
            # shards in ONE strided DMA (descriptor-gen cost, and the odd
            # load sits on the post-A2A critical tail)
            # scalar queue: the A2A-gated odd load must not block the sync
            # queue (scatters/out stores); scalar's attention exps are all
            # ahead of it in queue order.  Two halves so the first odd
            # matmuls start as soon as shards 0-3 land.
            def emit_cc_sb_loads(parity):
                cs = cc_sb[:].rearrange("p (kt c) -> p kt c", kt=KT)
                co = cc_out[parity][:].rearrange("(j r) c -> r j c", j=NCORES)
                half = NCORES // 2
                nc.scalar.dma_start(
                    cs[:, parity : parity + NCORES : 2, :], co[:, 0:half, :]
                )
                nc.scalar.dma_start(
                    cs[:, parity + NCORES :: 2, :], co[:, half:NCORES, :]
                )

            # the EVEN load is pinned on the sync queue behind pair-1's last
            # scatter: even-o_proj work is thereby RESERVED for the A2A-1
            # window (peer-skew + transfer, ~20us) instead of draining early
            # into attention-phase PE bubbles
            nc.sync.dma_start(
                cc_sb[:].rearrange("p (kt c) -> p kt c", kt=KT)[:, 0::2, :],
                cc_out[0][:].rearrange("(j r) c -> r j c", j=NCORES),
            )
            with (
                tc.tile_pool(name="ps_o", bufs=1, space="PSUM") as ps_o,
            ):
                # 8 PSUM banks, one per (n, m); even-kt chunks only need
                # cc_out[0] so this pass hides under the second AllToAll.
                o_ps = {}
                for n in range(NJC):
                    for m in range(2):
                        o_ps[(n, m)] = ps_o.tile(
                            [P, NQ], F32, tag=f"o{n}{m}", name=f"o{n}{m}"
                        )
                ot = None
                for parity in range(2):
                    for n in range(NJC):
                        for m in range(2):
                            for kk in range(KT // 2):
                                kt = 2 * kk + parity
                                nc.tensor.matmul(
                                    o_ps[(n, m)][:],
                                    cc_sb[
                                        :, kt * SROWS + P * m : kt * SROWS + P * (m + 1)
                                    ],
                                    wo_tiles[n][:, kt * NQ : (kt + 1) * NQ],
                                    start=(parity == 0 and kk == 0),
                                    stop=(parity == 1 and kk == KT // 2 - 1),
                                )
                            if parity == 1:
                                # both m row-tiles staged into one SBUF tile,
                                # stored with a single DMA per n
                                if m == 0:
                                    ot = outp.tile([P, 2 * NQ], BF16, tag="ot")
                                nc.scalar.copy(
                                    ot[:, NQ * m : NQ * (m + 1)], o_ps[(n, m)][:]
                                )
                                if m == 1:
                                    nc.sync.dma_start(
                                        out_d[:].rearrange(
                                            "(m r) c -> r m c", m=2
                                        )[:, :, NQ * n : NQ * (n + 1)],
                                        ot[:].rearrange("p (m c) -> p m c", m=2),
                                    )
                    if parity == 0:
                        emit_cc_sb_loads(1)

            wo_ctx.__exit__(None, None, None)

    nc.compile()
    return nc


def _get_nc():
    global _NC_CACHE
    if _NC_CACHE is None:
        _NC_CACHE = _build_nc()
    return _NC_CACHE


def _stage_inputs(position_ids, hidden_states, w_qkv, w_o):
    """Host-side sharding / layout staging. Returns in_maps for 8 cores."""
    pos = np.asarray(position_ids)[0].astype(np.float32)            # [S]
    hidden = np.asarray(hidden_states, dtype=np.float32)[0]         # [S, H]
    w_qkv = np.asarray(w_qkv, dtype=np.float32)                     # [3072, H]
    w_o = np.asarray(w_o, dtype=np.float32)                         # [H, Q_SIZE]

    # hT tiles: [H, S] -> [128, NJC*KT*NQ], n-major (query-chunk n's columns
    # of all kt tiles contiguous, so phase-1 loads are few big DMAs in
    # priority order)
    hT = np.ascontiguousarray(hidden.T)
    hT_r = np.ascontiguousarray(
        hT.reshape(KT, P, NJC, NQ).transpose(1, 2, 0, 3).reshape(P, KT * S)
    ).astype(BF16_NP)

    # w_o.T tiles: [Q_SIZE, H] -> [128, KT*H]
    woT = np.ascontiguousarray(w_o.T)
    woT_r = np.ascontiguousarray(
        woT.reshape(KT, P, H).transpose(1, 0, 2).reshape(P, KT * H)
    ).astype(BF16_NP)

    # rope tables in [d, s] layout for a [128 = 2 heads x 64] tile
    inv_freq = (1.0 / (ROPE_THETA ** (np.arange(0, HD, 2, dtype=np.float32) / HD)))
    ang = pos[:, None] * inv_freq[None, :]                          # [S, 32]
    cosT = np.cos(ang).T.astype(np.float32)                         # [32, S]
    sinT = np.sin(ang).T.astype(np.float32)
    ropec = np.concatenate([cosT, cosT, cosT, cosT], axis=0).astype(BF16_NP)
    ropes = np.concatenate([-sinT, sinT, -sinT, sinT], axis=0).astype(BF16_NP)

    # single [128, 128] causal triangle: within any diagonal 128-block,
    # key k is visible to query q iff k <= q
    kk = np.arange(P)
    masks = (kk[:, None] <= kk[None, :]).astype(BF16_NP)            # [128, 128]

    ident = np.eye(P, dtype=np.float32)

    in_maps = []
    for i in range(NCORES):
        rows_q = w_qkv[QH * HD * i : QH * HD * (i + 1)]             # [256, H]
        row_k = w_qkv[Q_SIZE + HD * i : Q_SIZE + HD * (i + 1)]      # [64, H]
        row_v = w_qkv[Q_SIZE + KV_SIZE + HD * i : Q_SIZE + KV_SIZE + HD * (i + 1)]
        wshard = np.concatenate([rows_q, row_k, row_v], axis=0)     # [384, H]
        wqkvT = np.ascontiguousarray(wshard.T)                      # [H, 384]
        wqkvT_r = np.ascontiguousarray(
            wqkvT.reshape(KT, P, 384).transpose(1, 0, 2).reshape(P, KT * 384)
        ).astype(BF16_NP)
        in_maps.append(
            {
                "hT": hT_r,
                "wqkvT": wqkvT_r,
                "woT": woT_r,
                "ropec": ropec,
                "ropes": ropes,
                "masks": masks,
                "ident": ident,
            }
        )
    return in_maps


def _ensure_ntff_hook():
    """The container's antenv stub lacks axon_hooks, so trn_boot silently
    skipped NTFF hook registration. Recreate the module and register the
    ctypes-based hook so run_bass_kernel_spmd(trace=True) can profile."""
    import sys
    import types

    if "antenv.axon_hooks" in sys.modules:
        return
    try:
        import antenv
        from trn_agent_boot.trn_boot import _ntff_profile_via_ctypes

        hooks = types.ModuleType("antenv.axon_hooks")
        _state = {}

        def set_axon_ntff_profile_hook(h):
            _state["h"] = h

        def get_axon_ntff_profile_hook():
            return _state.get("h")

        hooks.set_axon_ntff_profile_hook = set_axon_ntff_profile_hook
        hooks.get_axon_ntff_profile_hook = get_axon_ntff_profile_hook
        sys.modules["antenv.axon_hooks"] = hooks
        antenv.axon_hooks = hooks
        hook = _ntff_profile_via_ctypes("/opt/axon/libaxon_pjrt.so")
        if hook is not None:
            set_axon_ntff_profile_hook(hook)
    except Exception:
        pass


def kernel(**inputs):
    global LAST_RESULTS
    from concourse.bass_utils import run_bass_kernel_spmd

    nc = _get_nc()
    in_maps = _stage_inputs(
        inputs["position_ids"], inputs["hidden_states"], inputs["w_qkv"], inputs["w_o"]
    )
    trace = os.environ.get("KERNEL_TRACE", "0") == "1"
    if trace:
        _ensure_ntff_hook()
    res = run_bass_kernel_spmd(
        nc, in_maps, core_ids=list(range(NCORES)), trace=trace
    )
    LAST_RESULTS = res
    outs = [np.asarray(res.results[i]["out"], dtype=np.float32) for i in range(NCORES)]
    full = np.concatenate(outs, axis=0)                             # [S, H]
    return full.reshape(1, S, H)

